# revision 39
# baseline (speedup 1.0000x reference)
"""Multi-head attention (B=4, N=2048, C=1024, H=8, Dh=128) on 8 TRN2 NeuronCores.

Sharding: query-block parallel. Core c handles batch c//2, query-token half c%2
(1024 queries), all 8 heads. No device collectives: each core computes K/V for
its whole batch (2048 keys) locally. Host reorders tokens per core so the core's
own query half is always tokens [0:1024] (SPMD: all cores run one graph).

Math per core:
  QKV proj (float32r matmuls), scores = Q K^T (pre-scaled via weights),
  softmax via sampled-max-shift exp (ACT, fused bias/accum), PV with PE-transposed
  bf16 probs, out-proj with fused bias. Output y.T [1024 cout, 1024 tok].
"""

import sys

if "/opt/trn_rl_repo" not in sys.path:
    sys.path.insert(0, "/opt/trn_rl_repo")

from contextlib import ExitStack

import numpy as np

import concourse.bass as bass
import concourse.mybir as mybir
from concourse import bacc
from concourse.bass_utils import run_bass_kernel_spmd
from concourse.masks import make_identity
from concourse.tile import TileContext

F32 = mybir.dt.float32
BF16 = mybir.dt.bfloat16
FP16 = mybir.dt.float16
AF = mybir.ActivationFunctionType
ALU = mybir.AluOpType

DIM = 1024
HEADS = 8
HD = 128  # head dim
B, N = 4, 2048
SCALE = float(np.sqrt(DIM / HEADS))
NCORES = 8
TOK = 2048          # query tokens per core (whole batch)
KEYS = 2048         # keys per core (whole batch)
MARGIN = 60.0       # exp bias safety margin below sampled max
CLAMP = 1.0e30      # post-exp clamp (inf -> finite); legit values stay below


def _build():
    nc = bacc.Bacc("TRN2", target_bir_lowering=False, debug=False, num_devices=NCORES)

    # head-split sharding: each core owns HL=4 heads of one batch, all 2048
    # queries; the pair's partial output projections are summed on the host.
    xT_e = nc.declare_dram_parameter("xT", [2, 8, 128, 1024], FP16, isOutput=False)
    wqT_e = nc.declare_dram_parameter("wqT", [4, 8, 128, 128], FP16, isOutput=False)
    wkT_e = nc.declare_dram_parameter("wkT", [4, 8, 128, 128], FP16, isOutput=False)
    wvT_e = nc.declare_dram_parameter("wvT", [1, 8, 128, 512], FP16, isOutput=False)
    w0T_e = nc.declare_dram_parameter("w0T", [8, 4, 128, 128], FP16, isOutput=False)
    bq_e = nc.declare_dram_parameter("bq", [128, 4], F32, isOutput=False)
    b0_e = nc.declare_dram_parameter("b0", [128, 8], F32, isOutput=False)
    out_e = nc.declare_dram_parameter("out", [DIM, TOK], FP16, isOutput=True)
    rbounce = nc.dram_tensor("rbounce", [16, 4, 128], F32)
    HL = 4  # local heads per core

    with TileContext(nc) as tc, ExitStack() as ctx:
        persist = ctx.enter_context(tc.tile_pool(name="persist", bufs=1))
        QT = persist.tile([128, 4, TOK], FP16)         # [d, lhead, qtok]
        KT = persist.tile([128, 4, KEYS], FP16)        # [d, lhead, key]
        V = persist.tile([128, 16, 512], BF16)         # [tok%128, keytile, lfeat]
        bq_s = persist.tile([128, 4], F32)
        b0_s = persist.tile([128, 8], F32)
        ident = persist.tile([128, 128], BF16)
        ident_f32 = persist.tile([128, 128], F32)

        nc.gpsimd.dma_start(out=bq_s[:, :], in_=bq_e[:, :])
        nc.gpsimd.dma_start(out=b0_s[:, :], in_=b0_e[:, :])
        make_identity(nc, ident[:, :])
        make_identity(nc, ident_f32[:, :])

        # ---------------- QKV projection, two token-half phases ----------------
        with ExitStack() as qkv_ctx:
            xpool = qkv_ctx.enter_context(tc.tile_pool(name="xT", bufs=2))
            wp128 = qkv_ctx.enter_context(tc.tile_pool(name="w128", bufs=4))
            wp512 = qkv_ctx.enter_context(tc.tile_pool(name="w512", bufs=4))
            pq = qkv_ctx.enter_context(tc.tile_pool(name="pq", bufs=6, space="PSUM"))

            for ph in range(2):
                xt = xpool.tile([128, 8, 1024], FP16)
                if ph == 0:
                    # land the first weight tile on queue 0 before the x chunks
                    wq0 = wp128.tile([128, 8, 128], FP16, tag="w128")
                    nc.gpsimd.dma_start(out=wq0[:, :, :],
                                        in_=wqT_e[0].rearrange("c p f -> p c f"))
                for c in range(8):
                    nc.gpsimd.dma_start(out=xt[:, c, :], in_=xT_e[ph, c])

                if ph == 0:
                    wv0 = wp512.tile([128, 8, 512], FP16, tag="w512")
                    nc.gpsimd.dma_start(
                        out=wv0[:, :, :],
                        in_=wvT_e[0].rearrange("c p f -> p c f"))

                # Q projection for this half's queries
                for ft in range(4):
                    if ph == 0 and ft == 0:
                        wq = wq0
                    else:
                        wq = wp128.tile([128, 8, 128], FP16, tag="w128")
                        nc.gpsimd.dma_start(
                            out=wq[:, :, :],
                            in_=wqT_e[ft].rearrange("c p f -> p c f"))
                    for tch in range(2):
                        ps = pq.tile([128, 512], F32)
                        for c in range(8):
                            nc.tensor.matmul(
                                ps[:, :], wq[:, c, :],
                                xt[:, c, tch * 512:(tch + 1) * 512],
                                start=(c == 0), stop=(c == 7))
                        nc.scalar.activation(
                            QT[:, ft, ph * 1024 + tch * 512:
                               ph * 1024 + (tch + 1) * 512], ps[:, :],
                            AF.Identity, bias=bq_s[:, ft:ft + 1])

                # K projection for this half's keys
                for ft in range(4):
                    wk = wp128.tile([128, 8, 128], FP16, tag="w128")
                    nc.gpsimd.dma_start(
                        out=wk[:, :, :],
                        in_=wkT_e[ft].rearrange("c p f -> p c f"))
                    for tch in range(2):
                        ps = pq.tile([128, 512], F32)
                        for c in range(8):
                            nc.tensor.matmul(
                                ps[:, :], wk[:, c, :],
                                xt[:, c, tch * 512:(tch + 1) * 512],
                                start=(c == 0), stop=(c == 7))
                        nc.scalar.copy(
                            KT[:, ft, ph * 1024 + tch * 512:
                               ph * 1024 + (tch + 1) * 512],
                            ps[:, :])

                # V projection for this half's keys: [tok, lfeat] layout
                for tt in range(8):
                    ps = pq.tile([128, 512], F32)
                    for c in range(8):
                        nc.tensor.matmul(
                            ps[:, :], xt[:, c, tt * 128:(tt + 1) * 128],
                            wv0[:, c, :], start=(c == 0), stop=(c == 7))
                    nc.scalar.copy(V[:, ph * 8 + tt, :], ps[:, :])

        # ---------------- attention ----------------
        wpool2 = ctx.enter_context(tc.tile_pool(name="w0", bufs=6))
        ypool = ctx.enter_context(tc.tile_pool(name="y", bufs=3))
        otpool = ctx.enter_context(tc.tile_pool(name="ot", bufs=1))
        OT = otpool.tile([128, 4, TOK], FP16)      # [d, head(=dchunk), qtok]
        with ExitStack() as att_ctx:
            spool = att_ctx.enter_context(tc.tile_pool(name="sc", bufs=3, space="PSUM"))
            tpool = att_ctx.enter_context(tc.tile_pool(name="tp", bufs=1, space="PSUM"))
            opool = att_ctx.enter_context(tc.tile_pool(name="ov", bufs=1, space="PSUM"))
            upool = att_ctx.enter_context(tc.tile_pool(name="u", bufs=6))
            utpool = att_ctx.enter_context(tc.tile_pool(name="ut", bufs=3))
            rpool = att_ctx.enter_context(tc.tile_pool(name="rb", bufs=3))
            small = att_ctx.enter_context(tc.tile_pool(name="sm", bufs=16))
            ostg = att_ctx.enter_context(tc.tile_pool(name="ostg", bufs=3))

            for h in range(HL):
                for qg in range(4):
                    UT4 = utpool.tile([128, 16, 4, 128], BF16)
                    rc4 = small.tile([128, 4], F32, tag="rc4")
                    for qi in range(4):
                        qt = qg * 4 + qi
                        q_sl = QT[:, h, qt * 128:(qt + 1) * 128]

                        u = upool.tile([128, KEYS], BF16)
                        negb = small.tile([128, 1], F32, tag="negb")
                        ra = small.tile([128, 1], F32, tag="ra")
                        rb = small.tile([128, 1], F32, tag="rb")
                        ps_halves = []
                        for half in range(2):
                            ps = spool.tile([128, 1024], F32, tag="sc")
                            ps_halves.append(ps)
                            for kc in range(2):
                                nc.tensor.matmul(
                                    ps[:, kc * 512:(kc + 1) * 512], q_sl,
                                    KT[:, h, half * 1024 + kc * 512:
                                       half * 1024 + (kc + 1) * 512],
                                    start=True, stop=True)
                        mx = small.tile([128, 1], F32, tag="mx")
                        with tc.high_priority(offset=30):
                            nc.vector.tensor_reduce(
                                mx[:, :],
                                ps_halves[0][:, :].rearrange(
                                    "p (n s) -> p n s", s=4)[:, :, 0],
                                axis=mybir.AxisListType.X, op=ALU.max)
                        nc.gpsimd.tensor_scalar(
                            negb[:, :], mx[:, :], -1.0, -MARGIN,
                            op0=ALU.mult, op1=ALU.add)
                        for half in range(2):
                            nc.scalar.activation(
                                u[:, half * 1024:(half + 1) * 1024],
                                ps_halves[half][:, :],
                                AF.Exp, bias=negb[:, :], scale=1.0,
                                accum_out=(ra if half == 0 else rb)[:, :])
                            # transpose this half while the other half's exp runs;
                            # alternate psum banks so drains don't serialize
                            pool_t = tpool if half == 0 else opool
                            ps_t = pool_t.tile([128, 8, 128], BF16,
                                               tag="tp" if half == 0 else "ov")
                            for kt in range(8):
                                nc.tensor.transpose(
                                    ps_t[:, kt, :],
                                    u[:, (half * 8 + kt) * 128:
                                      (half * 8 + kt + 1) * 128],
                                    ident[:, :])
                            with tc.high_priority(offset=30):
                                nc.vector.tensor_scalar(
                                    UT4[:, half * 8:(half + 1) * 8, qi, :],
                                    ps_t[:, :, :], CLAMP, None, op0=ALU.min)

                        nc.gpsimd.tensor_add(ra[:, :], ra[:, :], rb[:, :])
                        nc.vector.reciprocal(rc4[:, qi:qi + 1], ra[:, :])


                    # transpose per-q reciprocals into a [1-per-free] row block,
                    # broadcast across partitions, then normalize during O drain
                    ps_o = opool.tile([128, 512], F32, tag="ov")
                    for kt in range(16):
                        nc.tensor.matmul(
                            ps_o[:, :], V[:, kt, h * 128:(h + 1) * 128],
                            UT4[:, kt, :, :].rearrange("p a b -> p (a b)"),
                            start=(kt == 0), stop=(kt == 15))
                    ps_r = opool.tile([4, 128], F32, tag="ov")
                    nc.tensor.transpose(ps_r[:, :], rc4[:, :], ident_f32[:, :])
                    rT = small.tile([4, 128], F32, tag="rT")
                    nc.vector.tensor_copy(rT[:, :], ps_r[:, :])
                    g = h * 4 + qg
                    nc.gpsimd.dma_start(out=rbounce[g, :, :], in_=rT[:, :])
                    rbc = rpool.tile([128, 4, 128], F32)
                    nc.gpsimd.dma_start(
                        out=rbc[:, :, :],
                        in_=bass.AP(tensor=rbounce[g].tensor,
                                    offset=rbounce[g].offset,
                                    ap=[[0, 128]] + list(rbounce[g].ap)))
                    # stage O out of PSUM promptly so the bank frees for the
                    # next group's PV; normalize asynchronously from the stage
                    o_stage = ostg.tile([128, 512], F32)
                    nc.vector.tensor_copy(o_stage[:, :], ps_o[:, :])
                    # last head's OT feeds outproj immediately: keep its
                    # normalize off the (slower) gpsimd queue
                    eng = nc.vector if h == HL - 1 else nc.gpsimd
                    eng.tensor_tensor(
                        out=OT[:, h, qg * 512:(qg + 1) * 512],
                        in0=o_stage[:, :].rearrange("p (a b) -> p a b", a=4),
                        in1=rbc[:, :, :], op=ALU.mult)

        # ---------------- output projection ----------------
        with ExitStack() as op_ctx:
            pyp = op_ctx.enter_context(tc.tile_pool(name="py", bufs=3, space="PSUM"))
            for ct in range(8):
                w0 = wpool2.tile([128, 4, 128], FP16)
                nc.gpsimd.dma_start(
                    out=w0[:, :, :],
                    in_=w0T_e[ct].rearrange("c p f -> p c f"))
                for tch in range(4):
                    ps = pyp.tile([128, 512], F32)
                    for dc in range(4):
                        nc.tensor.matmul(
                            ps[:, :], w0[:, dc, :],
                            OT[:, dc, tch * 512:(tch + 1) * 512],
                            start=(dc == 0), stop=(dc == 3))
                    y = ypool.tile([128, 512], FP16)
                    nc.vector.tensor_scalar(
                        y[:, :], ps[:, :], b0_s[:, ct:ct + 1], None, op0=ALU.add)
                    nc.gpsimd.dma_start(
                        out=out_e[ct * 128:(ct + 1) * 128,
                                  tch * 512:(tch + 1) * 512],
                        in_=y[:, :])

    nc.compile()
    return nc


_NC = None


def _get_nc():
    global _NC
    if _NC is None:
        _NC = _build()
    return _NC


def _make_in_maps(x, W_qkv, b_qkv, W0, b0):
    x = np.asarray(x, dtype=np.float32)
    W_qkv = np.asarray(W_qkv, dtype=np.float32)
    b_qkv = np.asarray(b_qkv, dtype=np.float32)
    W0 = np.asarray(W0, dtype=np.float32)
    b0 = np.asarray(b0, dtype=np.float32)

    def tile_w(wT, fsz):
        # [1024 cin, F] -> [F/fsz, 8, 128, fsz] contiguous
        nf = wT.shape[1] // fsz
        return np.ascontiguousarray(
            wT.reshape(8, 128, nf, fsz).transpose(2, 0, 1, 3)
        ).astype(np.float16)

    # V-bias folds through the output projection (softmax rows sum to 1);
    # K-bias only shifts each score row uniformly, which softmax cancels.
    # Each core of a pair adds half of the effective output bias.
    b0_eff = 0.5 * (b0 + W0 @ b_qkv[2 * DIM:3 * DIM])
    b0r = np.ascontiguousarray(b0_eff.reshape(8, 128).T).astype(np.float32)

    in_maps = []
    for c in range(NCORES):
        b, g = c // 2, c % 2
        hs = slice(g * 512, (g + 1) * 512)  # this core's 4 heads (features)
        wqT = tile_w((W_qkv[0:DIM] * SCALE).T[:, hs], 128)
        wkT = tile_w(W_qkv[DIM:2 * DIM].T[:, hs], 128)
        wvT = tile_w(W_qkv[2 * DIM:3 * DIM].T[:, hs], 512)
        # w0T rows for this head group: [512 din, 1024 cout] -> [8ct, 4c, 128, 128]
        w0T = np.ascontiguousarray(
            W0.T[g * 512:(g + 1) * 512].reshape(4, 128, 8, 128)
            .transpose(2, 0, 1, 3)).astype(np.float16)
        bq = np.ascontiguousarray(
            (b_qkv[0:DIM] * SCALE)[hs].reshape(4, 128).T).astype(np.float32)
        xT = np.ascontiguousarray(
            x[b].T.reshape(8, 128, 2, 1024).transpose(2, 0, 1, 3)
        ).astype(np.float16)
        in_maps.append({
            "xT": xT, "wqT": wqT, "wkT": wkT, "wvT": wvT, "w0T": w0T,
            "bq": bq, "b0": b0r,
        })
    return in_maps


def _assemble(results):
    y = np.empty((B, N, DIM), dtype=np.float32)
    for b in range(B):
        y[b] = (results[2 * b]["out"].astype(np.float32)
                + results[2 * b + 1]["out"].astype(np.float32)).T
    return y


def kernel(x, W_qkv, b_qkv, W0, b0):
    nc = _get_nc()
    in_maps = _make_in_maps(x, W_qkv, b_qkv, W0, b0)
    res = run_bass_kernel_spmd(nc, in_maps, core_ids=list(range(NCORES)))
    return _assemble(res.results)


def kernel_traced(x, W_qkv, b_qkv, W0, b0, tmpdir=None):
    """Same as kernel() but with NTFF profiling; returns (output, BassKernelResults)."""
    nc = _get_nc()
    in_maps = _make_in_maps(x, W_qkv, b_qkv, W0, b0)
    res = run_bass_kernel_spmd(nc, in_maps, core_ids=list(range(NCORES)),
                               trace=True, trace_cores=[0], tmpdir=tmpdir)
    return _assemble(res.results), res


# revision 40
# speedup vs baseline: 1.0136x; 1.0136x over previous
"""Multi-head attention (B=4, N=2048, C=1024, H=8, Dh=128) on 8 TRN2 NeuronCores.

Sharding: query-block parallel. Core c handles batch c//2, query-token half c%2
(1024 queries), all 8 heads. No device collectives: each core computes K/V for
its whole batch (2048 keys) locally. Host reorders tokens per core so the core's
own query half is always tokens [0:1024] (SPMD: all cores run one graph).

Math per core:
  QKV proj (float32r matmuls), scores = Q K^T (pre-scaled via weights),
  softmax via sampled-max-shift exp (ACT, fused bias/accum), PV with PE-transposed
  bf16 probs, out-proj with fused bias. Output y.T [1024 cout, 1024 tok].
"""

import sys

if "/opt/trn_rl_repo" not in sys.path:
    sys.path.insert(0, "/opt/trn_rl_repo")

from contextlib import ExitStack

import numpy as np

import concourse.bass as bass
import concourse.mybir as mybir
from concourse import bacc
from concourse.bass_utils import run_bass_kernel_spmd
from concourse.masks import make_identity
from concourse.tile import TileContext

F32 = mybir.dt.float32
BF16 = mybir.dt.bfloat16
FP16 = mybir.dt.float16
AF = mybir.ActivationFunctionType
ALU = mybir.AluOpType

DIM = 1024
HEADS = 8
HD = 128  # head dim
B, N = 4, 2048
SCALE = float(np.sqrt(DIM / HEADS))
NCORES = 8
TOK = 2048          # query tokens per core (whole batch)
KEYS = 2048         # keys per core (whole batch)
MARGIN = 60.0       # exp bias safety margin below sampled max
CLAMP = 1.0e30      # post-exp clamp (inf -> finite); legit values stay below


def _build():
    nc = bacc.Bacc("TRN2", target_bir_lowering=False, debug=False, num_devices=NCORES)

    # head-split sharding: each core owns HL=4 heads of one batch, all 2048
    # queries; the pair's partial output projections are summed on the host.
    xT_e = nc.declare_dram_parameter("xT", [2, 8, 128, 1024], FP16, isOutput=False)
    wqT_e = nc.declare_dram_parameter("wqT", [4, 8, 128, 128], FP16, isOutput=False)
    wkT_e = nc.declare_dram_parameter("wkT", [4, 8, 128, 128], FP16, isOutput=False)
    wvT_e = nc.declare_dram_parameter("wvT", [1, 8, 128, 512], FP16, isOutput=False)
    w0T_e = nc.declare_dram_parameter("w0T", [8, 4, 128, 128], FP16, isOutput=False)
    bq_e = nc.declare_dram_parameter("bq", [128, 4], F32, isOutput=False)
    b0_e = nc.declare_dram_parameter("b0", [128, 8], F32, isOutput=False)
    out_e = nc.declare_dram_parameter("out", [DIM, TOK], FP16, isOutput=True)
    rbounce = nc.dram_tensor("rbounce", [16, 4, 128], F32)
    HL = 4  # local heads per core

    with TileContext(nc) as tc, ExitStack() as ctx:
        persist = ctx.enter_context(tc.tile_pool(name="persist", bufs=1))
        QT = persist.tile([128, 4, TOK], FP16)         # [d, lhead, qtok]
        KT = persist.tile([128, 4, KEYS], FP16)        # [d, lhead, key]
        V = persist.tile([128, 16, 512], BF16)         # [tok%128, keytile, lfeat]
        bq_s = persist.tile([128, 4], F32)
        b0_s = persist.tile([128, 8], F32)
        ident = persist.tile([128, 128], BF16)
        ident_f32 = persist.tile([128, 128], F32)

        nc.gpsimd.dma_start(out=bq_s[:, :], in_=bq_e[:, :])
        nc.gpsimd.dma_start(out=b0_s[:, :], in_=b0_e[:, :])
        make_identity(nc, ident[:, :])
        make_identity(nc, ident_f32[:, :])

        # ---------------- QKV projection, two token-half phases ----------------
        with ExitStack() as qkv_ctx:
            xpool = qkv_ctx.enter_context(tc.tile_pool(name="xT", bufs=2))
            wp128 = qkv_ctx.enter_context(tc.tile_pool(name="w128", bufs=4))
            wp512 = qkv_ctx.enter_context(tc.tile_pool(name="w512", bufs=4))
            pq = qkv_ctx.enter_context(tc.tile_pool(name="pq", bufs=6, space="PSUM"))

            for ph in range(2):
                xt = xpool.tile([128, 8, 1024], FP16)
                if ph == 0:
                    # land the first weight tile on queue 0 before the x chunks
                    wq0 = wp128.tile([128, 8, 128], FP16, tag="w128")
                    nc.gpsimd.dma_start(out=wq0[:, :, :],
                                        in_=wqT_e[0].rearrange("c p f -> p c f"))
                for c in range(8):
                    nc.gpsimd.dma_start(out=xt[:, c, :], in_=xT_e[ph, c])

                if ph == 0:
                    wv0 = wp512.tile([128, 8, 512], FP16, tag="w512")
                    nc.gpsimd.dma_start(
                        out=wv0[:, :, :],
                        in_=wvT_e[0].rearrange("c p f -> p c f"))

                # Q projection for this half's queries
                for ft in range(4):
                    if ph == 0 and ft == 0:
                        wq = wq0
                    else:
                        wq = wp128.tile([128, 8, 128], FP16, tag="w128")
                        nc.gpsimd.dma_start(
                            out=wq[:, :, :],
                            in_=wqT_e[ft].rearrange("c p f -> p c f"))
                    for tch in range(2):
                        ps = pq.tile([128, 512], F32)
                        for c in range(8):
                            nc.tensor.matmul(
                                ps[:, :], wq[:, c, :],
                                xt[:, c, tch * 512:(tch + 1) * 512],
                                start=(c == 0), stop=(c == 7))
                        nc.scalar.activation(
                            QT[:, ft, ph * 1024 + tch * 512:
                               ph * 1024 + (tch + 1) * 512], ps[:, :],
                            AF.Identity, bias=bq_s[:, ft:ft + 1])

                # K projection for this half's keys
                for ft in range(4):
                    wk = wp128.tile([128, 8, 128], FP16, tag="w128")
                    nc.gpsimd.dma_start(
                        out=wk[:, :, :],
                        in_=wkT_e[ft].rearrange("c p f -> p c f"))
                    for tch in range(2):
                        ps = pq.tile([128, 512], F32)
                        for c in range(8):
                            nc.tensor.matmul(
                                ps[:, :], wk[:, c, :],
                                xt[:, c, tch * 512:(tch + 1) * 512],
                                start=(c == 0), stop=(c == 7))
                        nc.scalar.copy(
                            KT[:, ft, ph * 1024 + tch * 512:
                               ph * 1024 + (tch + 1) * 512],
                            ps[:, :])

                # V projection for this half's keys: [tok, lfeat] layout
                for tt in range(8):
                    ps = pq.tile([128, 512], F32)
                    for c in range(8):
                        nc.tensor.matmul(
                            ps[:, :], xt[:, c, tt * 128:(tt + 1) * 128],
                            wv0[:, c, :], start=(c == 0), stop=(c == 7))
                    nc.scalar.copy(V[:, ph * 8 + tt, :], ps[:, :])

        # ---------------- attention ----------------
        wpool2 = ctx.enter_context(tc.tile_pool(name="w0", bufs=6))
        ypool = ctx.enter_context(tc.tile_pool(name="y", bufs=3))
        otpool = ctx.enter_context(tc.tile_pool(name="ot", bufs=1))
        OT = otpool.tile([128, 4, TOK], FP16)      # [d, head(=dchunk), qtok]
        with ExitStack() as att_ctx:
            spool = att_ctx.enter_context(tc.tile_pool(name="sc", bufs=3, space="PSUM"))
            tpool = att_ctx.enter_context(tc.tile_pool(name="tp", bufs=1, space="PSUM"))
            opool = att_ctx.enter_context(tc.tile_pool(name="ov", bufs=1, space="PSUM"))
            upool = att_ctx.enter_context(tc.tile_pool(name="u", bufs=6))
            utpool = att_ctx.enter_context(tc.tile_pool(name="ut", bufs=3))
            rpool = att_ctx.enter_context(tc.tile_pool(name="rb", bufs=3))
            small = att_ctx.enter_context(tc.tile_pool(name="sm", bufs=16))
            ostg = att_ctx.enter_context(tc.tile_pool(name="ostg", bufs=3))

            for h in range(HL):
                for qg in range(4):
                    UT4 = utpool.tile([128, 16, 4, 128], BF16)
                    rc4 = small.tile([128, 4], F32, tag="rc4")
                    for qi in range(4):
                        qt = qg * 4 + qi
                        q_sl = QT[:, h, qt * 128:(qt + 1) * 128]

                        u = upool.tile([128, KEYS], BF16)
                        negb = small.tile([128, 1], F32, tag="negb")
                        ra = small.tile([128, 1], F32, tag="ra")
                        rb = small.tile([128, 1], F32, tag="rb")
                        ps_halves = []
                        for half in range(2):
                            ps = spool.tile([128, 1024], F32, tag="sc")
                            ps_halves.append(ps)
                            for kc in range(2):
                                nc.tensor.matmul(
                                    ps[:, kc * 512:(kc + 1) * 512], q_sl,
                                    KT[:, h, half * 1024 + kc * 512:
                                       half * 1024 + (kc + 1) * 512],
                                    start=True, stop=True)
                        mx = small.tile([128, 1], F32, tag="mx")
                        with tc.high_priority(offset=30):
                            nc.vector.tensor_reduce(
                                mx[:, :],
                                ps_halves[0][:, :].rearrange(
                                    "p (n s) -> p n s", s=4)[:, :, 0],
                                axis=mybir.AxisListType.X, op=ALU.max)
                        nc.gpsimd.tensor_scalar(
                            negb[:, :], mx[:, :], -1.0, -MARGIN,
                            op0=ALU.mult, op1=ALU.add)
                        for half in range(2):
                            nc.scalar.activation(
                                u[:, half * 1024:(half + 1) * 1024],
                                ps_halves[half][:, :],
                                AF.Exp, bias=negb[:, :], scale=1.0,
                                accum_out=(ra if half == 0 else rb)[:, :])
                            # transpose this half while the other half's exp runs;
                            # alternate psum banks so drains don't serialize
                            pool_t = tpool if half == 0 else opool
                            ps_t = pool_t.tile([128, 8, 128], BF16,
                                               tag="tp" if half == 0 else "ov")
                            for kt in range(8):
                                nc.tensor.transpose(
                                    ps_t[:, kt, :],
                                    u[:, (half * 8 + kt) * 128:
                                      (half * 8 + kt + 1) * 128],
                                    ident[:, :])
                            with tc.high_priority(offset=30):
                                nc.vector.tensor_scalar(
                                    UT4[:, half * 8:(half + 1) * 8, qi, :],
                                    ps_t[:, :, :], CLAMP, None, op0=ALU.min)

                        nc.gpsimd.tensor_add(ra[:, :], ra[:, :], rb[:, :])
                        nc.vector.reciprocal(rc4[:, qi:qi + 1], ra[:, :])


                    # transpose per-q reciprocals into a [1-per-free] row block,
                    # broadcast across partitions, then normalize during O drain
                    ps_o = opool.tile([128, 512], F32, tag="ov")
                    for kt in range(16):
                        nc.tensor.matmul(
                            ps_o[:, :], V[:, kt, h * 128:(h + 1) * 128],
                            UT4[:, kt, :, :].rearrange("p a b -> p (a b)"),
                            start=(kt == 0), stop=(kt == 15))
                    ps_r = opool.tile([4, 128], F32, tag="ov")
                    nc.tensor.transpose(ps_r[:, :], rc4[:, :], ident_f32[:, :])
                    rT = small.tile([4, 128], F32, tag="rT")
                    nc.vector.tensor_copy(rT[:, :], ps_r[:, :])
                    g = h * 4 + qg
                    nc.gpsimd.dma_start(out=rbounce[g, :, :], in_=rT[:, :])
                    rbc = rpool.tile([128, 4, 128], F32)
                    nc.gpsimd.dma_start(
                        out=rbc[:, :, :],
                        in_=bass.AP(tensor=rbounce[g].tensor,
                                    offset=rbounce[g].offset,
                                    ap=[[0, 128]] + list(rbounce[g].ap)))
                    # stage O out of PSUM promptly so the bank frees for the
                    # next group's PV; normalize asynchronously from the stage
                    o_stage = ostg.tile([128, 512], F32)
                    nc.vector.tensor_copy(o_stage[:, :], ps_o[:, :])
                    # last head's OT feeds outproj immediately: keep its
                    # normalize off the (slower) gpsimd queue
                    eng = nc.vector if h == HL - 1 else nc.gpsimd
                    eng.tensor_tensor(
                        out=OT[:, h, qg * 512:(qg + 1) * 512],
                        in0=o_stage[:, :].rearrange("p (a b) -> p a b", a=4),
                        in1=rbc[:, :, :], op=ALU.mult)

        # ---------------- output projection ----------------
        with ExitStack() as op_ctx:
            pyp = op_ctx.enter_context(tc.tile_pool(name="py", bufs=3, space="PSUM"))
            for ct in range(8):
                w0 = wpool2.tile([128, 4, 128], FP16)
                nc.gpsimd.dma_start(
                    out=w0[:, :, :],
                    in_=w0T_e[ct].rearrange("c p f -> p c f"))
                for tch in range(4):
                    ps = pyp.tile([128, 512], F32)
                    for dc in range(4):
                        nc.tensor.matmul(
                            ps[:, :], w0[:, dc, :],
                            OT[:, dc, tch * 512:(tch + 1) * 512],
                            start=(dc == 0), stop=(dc == 3))
                    y = ypool.tile([128, 512], FP16)
                    nc.scalar.activation(
                        y[:, :], ps[:, :], AF.Identity, bias=b0_s[:, ct:ct + 1])
                    nc.gpsimd.dma_start(
                        out=out_e[ct * 128:(ct + 1) * 128,
                                  tch * 512:(tch + 1) * 512],
                        in_=y[:, :])

    nc.compile()
    return nc


_NC = None


def _get_nc():
    global _NC
    if _NC is None:
        _NC = _build()
    return _NC


def _make_in_maps(x, W_qkv, b_qkv, W0, b0):
    x = np.asarray(x, dtype=np.float32)
    W_qkv = np.asarray(W_qkv, dtype=np.float32)
    b_qkv = np.asarray(b_qkv, dtype=np.float32)
    W0 = np.asarray(W0, dtype=np.float32)
    b0 = np.asarray(b0, dtype=np.float32)

    def tile_w(wT, fsz):
        # [1024 cin, F] -> [F/fsz, 8, 128, fsz] contiguous
        nf = wT.shape[1] // fsz
        return np.ascontiguousarray(
            wT.reshape(8, 128, nf, fsz).transpose(2, 0, 1, 3)
        ).astype(np.float16)

    # V-bias folds through the output projection (softmax rows sum to 1);
    # K-bias only shifts each score row uniformly, which softmax cancels.
    # Each core of a pair adds half of the effective output bias.
    b0_eff = 0.5 * (b0 + W0 @ b_qkv[2 * DIM:3 * DIM])
    b0r = np.ascontiguousarray(b0_eff.reshape(8, 128).T).astype(np.float32)

    in_maps = []
    for c in range(NCORES):
        b, g = c // 2, c % 2
        hs = slice(g * 512, (g + 1) * 512)  # this core's 4 heads (features)
        wqT = tile_w((W_qkv[0:DIM] * SCALE).T[:, hs], 128)
        wkT = tile_w(W_qkv[DIM:2 * DIM].T[:, hs], 128)
        wvT = tile_w(W_qkv[2 * DIM:3 * DIM].T[:, hs], 512)
        # w0T rows for this head group: [512 din, 1024 cout] -> [8ct, 4c, 128, 128]
        w0T = np.ascontiguousarray(
            W0.T[g * 512:(g + 1) * 512].reshape(4, 128, 8, 128)
            .transpose(2, 0, 1, 3)).astype(np.float16)
        bq = np.ascontiguousarray(
            (b_qkv[0:DIM] * SCALE)[hs].reshape(4, 128).T).astype(np.float32)
        xT = np.ascontiguousarray(
            x[b].T.reshape(8, 128, 2, 1024).transpose(2, 0, 1, 3)
        ).astype(np.float16)
        in_maps.append({
            "xT": xT, "wqT": wqT, "wkT": wkT, "wvT": wvT, "w0T": w0T,
            "bq": bq, "b0": b0r,
        })
    return in_maps


def _assemble(results):
    y = np.empty((B, N, DIM), dtype=np.float32)
    for b in range(B):
        y[b] = (results[2 * b]["out"].astype(np.float32)
                + results[2 * b + 1]["out"].astype(np.float32)).T
    return y


def kernel(x, W_qkv, b_qkv, W0, b0):
    nc = _get_nc()
    in_maps = _make_in_maps(x, W_qkv, b_qkv, W0, b0)
    res = run_bass_kernel_spmd(nc, in_maps, core_ids=list(range(NCORES)))
    return _assemble(res.results)


def kernel_traced(x, W_qkv, b_qkv, W0, b0, tmpdir=None):
    """Same as kernel() but with NTFF profiling; returns (output, BassKernelResults)."""
    nc = _get_nc()
    in_maps = _make_in_maps(x, W_qkv, b_qkv, W0, b0)
    res = run_bass_kernel_spmd(nc, in_maps, core_ids=list(range(NCORES)),
                               trace=True, trace_cores=[0], tmpdir=tmpdir)
    return _assemble(res.results), res
